# revision 18
# baseline (speedup 1.0000x reference)
"""MLA attention kernel for 8 TRN2 NeuronCores.

Sharding: core i handles batch b=i//4, heads h in [4*(i%4), 4*(i%4)+4).
Each head writes a disjoint 128-col slice of the output (the reference's
output einsum shares `h` between attention heads and output-channel
blocks), so no collective is needed: pure SPMD + host concat.

Math per core (batch b, 4 heads), equivalent to the reference modulo
fp reassociation:
  c_q  = x_b @ W_dq.T                 [T, 512]
  c_kv = x_b @ W_dkv.T                [T, 512]
  k_r  = rope(x_b @ W_kr.T)           [T, 64]   (shared by heads)
  per head h:
    q_h  = c_q @ A_h / sqrt(192)      [T, 128]  A_h = W_uq.reshape(512,16,128)[:,h,:]
    k_h  = c_kv @ B_h.T               [T, 128]  B_h = W_uk.reshape(16,128,512)[h]
    q_r  = rope(c_q @ W_qr_h.T)/sqrt  [T, 64]
    v_eff= W_uv.T @ W_o[h-block].T    [512, 128]
    v_h  = c_kv @ v_eff               [T, 128]
    S    = q_h k_h^T + q_r k_r^T  (causal)
    y_h  = softmax(S) @ v_h  ->  out[:, h*128:(h+1)*128]

On-chip layout: scores computed transposed S_T[s, t] so the exp output
P_T[s, t] feeds the PV matmul directly as the stationary operand
(contraction over s = partitions).  Row sums for softmax normalization
come from a ones-column appended to v_h (PV output col 128).  Scores
are O(1) here so exp needs no max-subtraction.  All inputs are
pre-arranged on the host into the exact SBUF tile layouts (partition-
major) so every DMA moves contiguous 16KB-per-partition runs.
"""

import sys

sys.path.insert(0, "/opt/trn_rl_repo")

import numpy as np
import ml_dtypes
from contextlib import ExitStack

import concourse.bass as bass
import concourse.bacc as bacc
import concourse.mybir as mybir
import concourse.tile as tile
from concourse.bass_utils import run_bass_kernel_spmd

B, T, C = 2, 2048, 2048
NH, HS = 16, 128
NLQ, NLKV, DHR = 512, 512, 64
HPC = 4  # heads per core
NCORES = 8
SCALE = 1.0 / float(np.sqrt(HS + DHR))

bf16 = ml_dtypes.bfloat16
F32 = mybir.dt.float32
B16 = mybir.dt.bfloat16
Copy = mybir.ActivationFunctionType.Copy
Exp = mybir.ActivationFunctionType.Exp

NT = T // 512  # 4 t-chunks
NS = T // 128  # 16 s-tiles
PV_LAG = 3  # S/exp runs this many s-tiles ahead of PV


def build():
    nc = bacc.Bacc("TRN2", target_bir_lowering=False, debug=False, num_devices=NCORES)

    xp = nc.dram_tensor("xp", [128, NT, 16, 512], B16, kind="ExternalInput")
    wdq = nc.dram_tensor("wdq", [128, 16, NLQ], B16, kind="ExternalInput")
    wdkv = nc.dram_tensor("wdkv", [128, 16, NLKV], B16, kind="ExternalInput")
    wkr = nc.dram_tensor("wkr", [128, 16, DHR], B16, kind="ExternalInput")
    wqr = nc.dram_tensor("wqr", [HPC, 128, 4, DHR], B16, kind="ExternalInput")
    A = nc.dram_tensor("A", [HPC, 128, 4, HS], B16, kind="ExternalInput")
    BT = nc.dram_tensor("BT", [HPC, 128, 4, HS], B16, kind="ExternalInput")
    wuv = nc.dram_tensor("wuv", [128, 16, NLKV], B16, kind="ExternalInput")
    woT = nc.dram_tensor("woT", [128, 16, HPC * HS], B16, kind="ExternalInput")
    cosT = nc.dram_tensor("cosT", [DHR // 2, T], B16, kind="ExternalInput")
    sinT = nc.dram_tensor("sinT", [DHR // 2, T], B16, kind="ExternalInput")
    masks = nc.dram_tensor("masks", [128, 4, 512], B16, kind="ExternalInput")
    out = nc.dram_tensor("out", [T, HPC * HS], F32, kind="ExternalOutput")

    with tile.TileContext(nc) as tc, ExitStack() as ctx:
        wpool = ctx.enter_context(tc.tile_pool(name="wpool", bufs=1))
        xpool = ctx.enter_context(tc.tile_pool(name="xpool", bufs=1))
        cpool = ctx.enter_context(tc.tile_pool(name="cpool", bufs=1))
        hwpool = ctx.enter_context(tc.tile_pool(name="hwpool", bufs=2))
        hpool = ctx.enter_context(tc.tile_pool(name="hpool", bufs=2))
        ppool = ctx.enter_context(tc.tile_pool(name="ppool", bufs=5))
        tpool = ctx.enter_context(tc.tile_pool(name="tpool", bufs=2))
        opool = ctx.enter_context(tc.tile_pool(name="opool", bufs=4))
        psA = ctx.enter_context(tc.tile_pool(name="psA", bufs=2, space="PSUM"))
        psS = ctx.enter_context(tc.tile_pool(name="psS", bufs=2, space="PSUM"))
        psY = ctx.enter_context(tc.tile_pool(name="psY", bufs=4, space="PSUM"))

        # ---- persistent weight loads (contiguous per-partition runs) ----
        wdq_sb = wpool.tile([128, 16, NLQ], B16)
        for g in range(4):
            nc.sync.dma_start(
                out=wdq_sb[:, g * 4 : (g + 1) * 4, :], in_=wdq.ap()[:, g * 4 : (g + 1) * 4]
            )
        wdkv_sb = wpool.tile([128, 16, NLKV], B16)
        nc.sync.dma_start(out=wdkv_sb[:], in_=wdkv.ap())
        wkr_sb = wpool.tile([128, 16, DHR], B16)
        nc.sync.dma_start(out=wkr_sb[:], in_=wkr.ap())
        cos_sb = wpool.tile([32, T], B16)
        nc.sync.dma_start(out=cos_sb[:], in_=cosT.ap())
        sin_sb = wpool.tile([32, T], B16)
        nc.sync.dma_start(out=sin_sb[:], in_=sinT.ap())

        # ---- prologue: c_q, c_kv, k_r ----
        cq_sb = cpool.tile([128, 4, T], B16)
        ckv_sb = cpool.tile([128, 4, T], B16)
        kr_cat_sb = cpool.tile([64, T], B16)

        for j in range(NT):
            t0 = j * 512
            xsls = []
            for g in range(4):
                xg = xpool.tile([128, 4, 512], B16, tag=f"xsl{g}", name=f"xsl_{j}_{g}")
                nc.scalar.dma_start(out=xg[:], in_=xp.ap()[:, j, g * 4 : (g + 1) * 4])
                xsls.append(xg)
            def xsl_at(ct):
                return xsls[ct // 4][:, ct % 4, :]
            for qt in range(4):
                ps = psA.tile([128, 512], F32, tag="psproj")
                for ct in range(16):
                    nc.tensor.matmul(
                        ps[:],
                        wdq_sb[:, ct, qt * 128 : (qt + 1) * 128],
                        xsl_at(ct),
                        start=(ct == 0),
                        stop=(ct == 15),
                    )
                nc.scalar.activation(cq_sb[:, qt, t0 : t0 + 512], ps[:], Copy)
            for qt in range(4):
                ps = psA.tile([128, 512], F32, tag="psproj")
                for ct in range(16):
                    nc.tensor.matmul(
                        ps[:],
                        wdkv_sb[:, ct, qt * 128 : (qt + 1) * 128],
                        xsl_at(ct),
                        start=(ct == 0),
                        stop=(ct == 15),
                    )
                nc.vector.tensor_copy(ckv_sb[:, qt, t0 : t0 + 512], ps[:])
            # k_r projection then rope
            ps = psA.tile([128, 512], F32, tag="psproj")
            for ct in range(16):
                nc.tensor.matmul(
                    ps[0:64, :],
                    wkr_sb[:, ct, :],
                    xsl_at(ct),
                    start=(ct == 0),
                    stop=(ct == 15),
                )
            _rope(nc, tpool, ps, kr_cat_sb, t0, cos_sb, sin_sb)

        # deferred loads (not needed until after the prologue starts)
        wuv_sb = wpool.tile([128, 16, NLKV], B16)
        nc.sync.dma_start(out=wuv_sb[:], in_=wuv.ap())
        mask_sb = wpool.tile([128, 4, 512], B16)
        nc.sync.dma_start(out=mask_sb[:], in_=masks.ap())
        woh_sb = wpool.tile([128, 16, HPC * HS], B16)
        nc.sync.dma_start(out=woh_sb[:], in_=woT.ap())

        # ---- shared V phase: all 4 heads at once (N=512 matmuls) ----
        # veff_all[k, (h,d)] = sum_c W_uv[c,k] * W_o[h-block].T[c,d]
        veff_sb = cpool.tile([128, 4, HPC * HS], B16)
        for kt in range(4):
            ps = psA.tile([128, 512], F32, tag="psproj")
            for ct in range(16):
                nc.tensor.matmul(
                    ps[:],
                    wuv_sb[:, ct, kt * 128 : (kt + 1) * 128],
                    woh_sb[:, ct, :],
                    start=(ct == 0),
                    stop=(ct == 15),
                )
            nc.vector.tensor_copy(veff_sb[:, kt, :], ps[:])
        # v_aug[s, (h, d|1)]: v for all heads + ones column per head
        vaug_sb = cpool.tile([128, NS, HPC, 132], B16)
        for st in range(NS):
            ps = psA.tile([128, 512], F32, tag="psproj")
            for kt in range(4):
                nc.tensor.matmul(
                    ps[:],
                    ckv_sb[:, kt, st * 128 : (st + 1) * 128],
                    veff_sb[:, kt, :],
                    start=(kt == 0),
                    stop=(kt == 3),
                )
            nc.vector.tensor_copy(
                vaug_sb[:, st, :, 0:128],
                ps[:].rearrange("p (h d) -> p h d", h=HPC),
            )
            nc.vector.memset(vaug_sb[:, st, :, 128:129], 1.0)

        # ---- per-head ----
        for h in range(HPC):
            a_sb = hwpool.tile([128, 4, HS], B16, tag="a_sb")
            nc.gpsimd.dma_start(out=a_sb[:], in_=A.ap()[h])
            bt_sb = hwpool.tile([128, 4, HS], B16, tag="bt_sb")
            nc.gpsimd.dma_start(out=bt_sb[:], in_=BT.ap()[h])
            wqr_sb = hwpool.tile([128, 4, DHR], B16, tag="wqr_sb")
            nc.gpsimd.dma_start(out=wqr_sb[:], in_=wqr.ap()[h])

            # q_hT/k_hT [128, T], roped q_r [64, T]
            qh_sb = hpool.tile([128, T], B16, tag="qh")
            kh_sb = hpool.tile([128, T], B16, tag="kh")
            qr_cat_sb = hpool.tile([64, T], B16, tag="qr_cat")
            for j in range(NT):
                t0 = j * 512
                ps = psA.tile([128, 512], F32, tag="psproj")
                for qt in range(4):
                    nc.tensor.matmul(
                        ps[:],
                        a_sb[:, qt, :],
                        cq_sb[:, qt, t0 : t0 + 512],
                        start=(qt == 0),
                        stop=(qt == 3),
                    )
                nc.scalar.activation(qh_sb[:, t0 : t0 + 512], ps[:], Copy)
                ps = psA.tile([128, 512], F32, tag="psproj")
                for kt in range(4):
                    nc.tensor.matmul(
                        ps[:],
                        bt_sb[:, kt, :],
                        ckv_sb[:, kt, t0 : t0 + 512],
                        start=(kt == 0),
                        stop=(kt == 3),
                    )
                nc.vector.tensor_copy(kh_sb[:, t0 : t0 + 512], ps[:])
                ps = psA.tile([128, 512], F32, tag="psproj")
                for qt in range(4):
                    nc.tensor.matmul(
                        ps[0:64, :],
                        wqr_sb[:, qt, :],
                        cq_sb[:, qt, t0 : t0 + 512],
                        start=(qt == 0),
                        stop=(qt == 3),
                    )
                _rope(nc, tpool, ps, qr_cat_sb, t0, cos_sb, sin_sb)

            # attention: for each t-chunk, accumulate over causal s-tiles.
            # S/exp runs PV_LAG s-tiles ahead of PV so the PE stream does
            # not stall on psY slot release at chunk boundaries.
            for j in range(NT):
                t0 = j * 512
                ys = [
                    psY.tile([128, 132], F32, tag="psy", name=f"psy_{h}_{j}_{m}")
                    for m in range(4)
                ]
                n_st = 4 * j + 4
                pts = {}

                def s_exp(i, h=h, j=j, t0=t0):
                    ss = psS.tile([128, 512], F32, tag="ps_s")
                    nc.tensor.matmul(
                        ss[:],
                        kh_sb[:, i * 128 : (i + 1) * 128],
                        qh_sb[:, t0 : t0 + 512],
                        start=True,
                        stop=False,
                    )
                    nc.tensor.matmul(
                        ss[:],
                        kr_cat_sb[:, i * 128 : (i + 1) * 128],
                        qr_cat_sb[:, t0 : t0 + 512],
                        start=False,
                        stop=True,
                    )
                    pt = ppool.tile([128, 512], B16, tag="pt", name=f"pt_{h}_{j}_{i}")
                    nc.scalar.activation(pt[:], ss[:], Exp)
                    m2 = i - 4 * j
                    if m2 >= 0:
                        nc.vector.tensor_mul(pt[:], pt[:], mask_sb[:, m2, :])
                    return pt

                def pv(i, ys=ys, pts=pts, n_st=n_st):
                    for m in range(4):
                        nc.tensor.matmul(
                            ys[m][:, 0:129],
                            pts[i][:, m * 128 : (m + 1) * 128],
                            vaug_sb[:, i, h, 0:129],
                            start=(i == 0),
                            stop=(i == n_st - 1),
                        )

                for i in range(n_st):
                    pts[i] = s_exp(i)
                    if i >= PV_LAG:
                        pv(i - PV_LAG)
                for i in range(n_st - PV_LAG, n_st):
                    pv(i)

                for m in range(4):
                    recip = tpool.tile([128, 1], F32, tag="recip")
                    nc.vector.reciprocal(recip[:], ys[m][:, 128:129])
                    o_sb = opool.tile([128, HS], F32, tag="o_sb")
                    nc.vector.tensor_scalar_mul(o_sb[:], ys[m][:, 0:128], recip[:])
                    nc.scalar.dma_start(
                        out=out.ap()[
                            t0 + m * 128 : t0 + (m + 1) * 128,
                            h * HS : (h + 1) * HS,
                        ],
                        in_=o_sb[:],
                    )

    nc.compile()
    return nc


def _rope(nc, tpool, ps, cat_sb, t0, cos_sb, sin_sb):
    """ps[0:64, :512] holds the projected (re|im col-permuted) vectors.
    Write roped bf16 into cat_sb[0:64, t0:t0+512].  DVE tensor_tensor
    requires all operands at the same start partition, so the im half is
    staged through base-0 tiles with ACT copies (ACT allows cross-base)."""
    cs = cos_sb[:, t0 : t0 + 512]
    sn = sin_sb[:, t0 : t0 + 512]
    im_sb = tpool.tile([32, 512], B16, tag="imsrc")
    nc.scalar.activation(im_sb[:], ps[32:64, :], Copy)
    t1 = tpool.tile([32, 512], B16, tag="ropet1")
    t2 = tpool.tile([32, 512], B16, tag="ropet2")
    nc.vector.tensor_mul(t1[:], ps[0:32, :], cs)
    nc.vector.tensor_mul(t2[:], im_sb[:], sn)
    nc.vector.tensor_sub(cat_sb[0:32, t0 : t0 + 512], t1[:], t2[:])
    t3 = tpool.tile([32, 512], B16, tag="ropet3")
    t4 = tpool.tile([32, 512], B16, tag="ropet4")
    nc.vector.tensor_mul(t3[:], ps[0:32, :], sn)
    nc.vector.tensor_mul(t4[:], im_sb[:], cs)
    im_ro = tpool.tile([32, 512], B16, tag="imrope")
    nc.vector.tensor_add(im_ro[:], t3[:], t4[:])
    nc.scalar.activation(cat_sb[32:64, t0 : t0 + 512], im_ro[:], Copy)


_NC_CACHE = {}


def _get_nc():
    if "nc" not in _NC_CACHE:
        _NC_CACHE["nc"] = build()
    return _NC_CACHE["nc"]


def _part_major(a, pt=128):
    """[pt*n, ...] -> [128, n, ...] partition-major contiguous."""
    n = a.shape[0] // pt
    return np.ascontiguousarray(
        a.reshape(n, pt, *a.shape[1:]).transpose(1, 0, *range(2, a.ndim + 1))
    )


def _prep_in_maps(x, cos, sin, W_dq, W_uq, W_dkv, W_uk, W_uv, W_qr, W_kr, W_o):
    perm = np.concatenate([np.arange(0, DHR, 2), np.arange(1, DHR, 2)])

    shared = {
        "wdq": _part_major(np.ascontiguousarray(W_dq.T).astype(bf16)),
        "wdkv": _part_major(np.ascontiguousarray(W_dkv.T).astype(bf16)),
        "wkr": _part_major(np.ascontiguousarray(W_kr.T[:, perm]).astype(bf16)),
        "wuv": _part_major(np.ascontiguousarray(W_uv).astype(bf16)),
        "cosT": np.ascontiguousarray(cos.T).astype(bf16),
        "sinT": np.ascontiguousarray(sin.T).astype(bf16),
    }
    p = np.arange(128)[:, None]
    f = np.arange(512)[None, :]
    m = np.stack([(p <= f - 128 * mm) for mm in range(4)], axis=1).astype(bf16)
    shared["masks"] = np.ascontiguousarray(m)  # [128, 4, 512]

    # x[b] -> xp[p, j, ct, f] = x[b][j*512+f, ct*128+p]
    xps = []
    for b in range(B):
        xb = np.asarray(x[b]).astype(bf16)  # [T, C]
        xps.append(
            np.ascontiguousarray(xb.reshape(NT, 512, 16, 128).transpose(3, 0, 2, 1))
        )

    A_full = np.asarray(W_uq).reshape(NLQ, NH, HS)
    B_full = np.asarray(W_uk).reshape(NH, HS, NLKV)

    head_maps = []
    for g in range(4):
        hs = [4 * g + i for i in range(HPC)]
        A_np = np.stack(
            [_part_major((A_full[:, h, :] * SCALE).astype(bf16)) for h in hs]
        )
        BT_np = np.stack([_part_major(np.ascontiguousarray(B_full[h].T).astype(bf16)) for h in hs])
        wqr_np = np.stack(
            [
                _part_major(
                    np.ascontiguousarray(
                        W_qr[h * DHR : (h + 1) * DHR, :].T[:, perm] * SCALE
                    ).astype(bf16)
                )
                for h in hs
            ]
        )
        # [128(c p), 16(ct), HPC*HS] with free = (h, d)
        woT_np = np.stack(
            [
                _part_major(np.ascontiguousarray(W_o[h * HS : (h + 1) * HS, :].T).astype(bf16))
                for h in hs
            ],
            axis=2,
        ).reshape(128, 16, HPC * HS)
        head_maps.append(
            {
                "A": np.ascontiguousarray(A_np),
                "BT": np.ascontiguousarray(BT_np),
                "wqr": np.ascontiguousarray(wqr_np),
                "woT": np.ascontiguousarray(woT_np),
            }
        )

    in_maps = []
    for core in range(NCORES):
        b, g = core // 4, core % 4
        im = dict(shared)
        im["xp"] = xps[b]
        im.update(head_maps[g])
        in_maps.append(im)
    return in_maps


def kernel_run(inputs, trace=False, trace_kwargs=None):
    nc = _get_nc()
    in_maps = _prep_in_maps(**{k: np.asarray(v) for k, v in inputs.items()})
    res = run_bass_kernel_spmd(
        nc,
        in_maps,
        core_ids=list(range(NCORES)),
        trace=trace,
        **(trace_kwargs or {}),
    )
    y = np.empty((B, T, C), np.float32)
    for core in range(NCORES):
        b, g = core // 4, core % 4
        y[b][:, g * 512 : (g + 1) * 512] = res.results[core]["out"]
    return y, res


def kernel(**inputs):
    y, _ = kernel_run(inputs)
    return y
